# revision 7
# baseline (speedup 1.0000x reference)
"""Trainium2 Bass kernel for CTM segment_reduce (map2token + token2map).

kernel(**inputs) takes the full arrays, shards batch B=8 across the 8
NeuronCores (one batch element per core), runs a Bass/Tile kernel, and
gathers the full outputs.

Math (H=W=H_init=W_init=56 makes the grid-index map the identity), per
batch element b with idx = idx_token[b] in [0,784)^3136:
  tokens[b, n, c] = sum_{i: idx[i]=n} fm[b, c, i] / (count_n + 1e-6)
  fmap[b, c, i]   = x[b, idx[i], c] / (1 + 1e-6)

Device strategy per core:
  - tokens (scatter-add): one-hot matmul on TensorE. One-hot tiles
    OH[i,n] = (idx[i]==n) are built on-chip (iota + is_equal) in fp16;
    fm arrives pre-transposed and split hi/lo in fp16 (hi+lo sums are
    exact to ~2^-22 relative; fp32r was measured to round to 12 mantissa
    bits, too lossy). Counts ride along as a ones column in the lo pass;
    PSUM (fp32) accumulates over all 25 i-tiles.
  - fmap (gather): indirect DMA row-gather of x (exact fp32) +
    PE transpose-mode to produce the c-major output layout.
"""

import copy
import os
import sys

import numpy as np

for _p in ("/opt/trn_rl_repo", "/root/.axon_site/_ro/trn_rl_repo"):
    if os.path.isdir(_p) and _p not in sys.path:
        sys.path.append(_p)

import concourse.bass as bass
import concourse.mybir as mybir
from concourse.masks import make_identity
from concourse.tile import TileContext

F32 = mybir.dt.float32
F16 = mybir.dt.float16
I32 = mybir.dt.int32

# Problem constants (hardcoded per contract)
B = 8
C = 256
HW = 3136           # 56*56 positions
N = 784             # tokens
HWP = 3200          # positions padded to 25*128
KI = 25             # i-tiles of 128
MT = [128, 128, 128, 128, 128, 128, 16]   # n-tile sizes (784 = 6*128+16)
MSTART = [0, 128, 256, 384, 512, 640, 768]
SENTINEL = 9999.0
INV_EPS1 = float(np.float32(1.0) / np.float32(1.0 + 1e-6))
GCH = 5             # gather i-tiles per indirect DMA

_CACHE = {}


def legalize_multiwait(nc):
    """This walrus build allows a limited number of sync waits per
    instruction; split extras onto single-wait EventSemaphore carriers
    inserted just before the offending instruction (same engine)."""
    template = None
    for f in nc.m.functions:
        for b in f.blocks:
            for ins in b.instructions:
                if type(ins).__name__ == "InstEventSemaphore":
                    template = ins
                    break
            if template:
                break
        if template:
            break
    assert template is not None, "no EventSemaphore template found"
    cnt = 0
    for f in nc.m.functions:
        for b in f.blocks:
            insts = list(b.instructions)
            out = []
            changed = False
            for ins in insts:
                si = ins.sync_info
                if si is not None and si.on_wait and len(si.on_wait) > 1:
                    waits = list(si.on_wait)
                    for w in waits[:-1]:
                        nop = copy.deepcopy(template)
                        nop.name = f"waitsplit_{cnt}"
                        cnt += 1
                        nop.engine = ins.engine
                        nop.sync_info = mybir.SyncInfo(on_wait=[w], on_update=[])
                        try:
                            nop.set_dependency_edges([])
                        except Exception:
                            pass
                        nc.register_instruction(nop, overwrite=True)
                        out.append(nop)
                    ins.sync_info = mybir.SyncInfo(
                        on_wait=[waits[-1]], on_update=list(si.on_update or [])
                    )
                    changed = True
                out.append(ins)
            if changed:
                b.instructions.clear()
                for i in out:
                    b.add_instruction(i)
    return cnt


def build_program():
    nc = bass.Bass("TRN2", target_bir_lowering=False, debug=False, num_swdge_queues=4)

    fthi_in = nc.declare_dram_parameter("fthi", [HWP, C], F16, isOutput=False)
    ftlo_in = nc.declare_dram_parameter("ftlo", [HWP, C + 1], F16, isOutput=False)
    xs_in = nc.declare_dram_parameter("xs", [N, C], F32, isOutput=False)
    idxT_in = nc.declare_dram_parameter("idxT", [128, KI], F32, isOutput=False)
    idxI_in = nc.declare_dram_parameter("idxI", [128, KI], I32, isOutput=False)
    tok_out = nc.declare_dram_parameter("tok", [N, C], F32, isOutput=True)
    fmo_out = nc.declare_dram_parameter("fmo", [2, 128, HW], F32, isOutput=True)

    with TileContext(nc) as tc:
        _emit(nc, tc, fthi_in, ftlo_in, xs_in, idxT_in, idxI_in, tok_out, fmo_out)
    legalize_multiwait(nc)
    return nc


def _emit(nc, tc, fthi_in, ftlo_in, xs_in, idxT_in, idxI_in, tok_out, fmo_out):
    from contextlib import ExitStack

    ctx = ExitStack()
    with ctx:
        const = ctx.enter_context(tc.tile_pool(name="const", bufs=1))
        big = ctx.enter_context(tc.tile_pool(name="big", bufs=1))
        tokL_pool = ctx.enter_context(tc.tile_pool(name="tokL", bufs=2))
        epi_pool = ctx.enter_context(tc.tile_pool(name="epi", bufs=2))
        psT = ctx.enter_context(tc.tile_pool(name="psT", bufs=2, space="PSUM"))
        psA = ctx.enter_context(tc.tile_pool(name="psA", bufs=1, space="PSUM"))

        # ---- input DMAs ----
        fthi = big.tile([128, KI, C], F16, tag="fthi")
        nc.sync.dma_start(
            out=fthi, in_=fthi_in[:, :].rearrange("(k p) c -> p k c", k=KI, p=128)
        )
        ftlo = big.tile([128, KI, C + 1], F16, tag="ftlo")
        nc.sync.dma_start(
            out=ftlo, in_=ftlo_in[:, :].rearrange("(k p) c -> p k c", k=KI, p=128)
        )
        idxTs = const.tile([128, KI], F32, tag="idxTs")
        nc.sync.dma_start(out=idxTs, in_=idxT_in[:, :])
        idxIs = const.tile([128, KI], I32, tag="idxIs")
        nc.sync.dma_start(out=idxIs, in_=idxI_in[:, :])

        # ---- constants ----
        ident = const.tile([128, 128], F32, tag="ident")
        make_identity(nc, ident)
        iotaN_i = const.tile([128, N], I32, tag="iotaN_i")
        nc.gpsimd.iota(iotaN_i, pattern=[[1, N]], base=0, channel_multiplier=0)
        iotaN = const.tile([128, N], F32, tag="iotaN")
        nc.gpsimd.tensor_copy(iotaN, iotaN_i)

        # ---- one-hot tiles OH[i, n] (fp16; all 25 kept resident) ----
        ohA = []
        for k in range(KI):
            oh = const.tile([128, N], F16, tag=f"ohA{k}", name=f"ohA{k}")
            eng = nc.vector if k % 2 else nc.gpsimd
            eng.tensor_scalar(
                oh, iotaN, idxTs[:, k:k + 1], None, op0=mybir.AluOpType.is_equal
            )
            ohA.append(oh)

        # ---- phase B: gather rows of x (exact f32) + PE transpose ----
        # HW honors only one offset per partition per indirect DMA -> 25
        # single-column gathers (128 rows x 1KB each)
        gth = big.tile([128, KI, C], F32, tag="gth")
        for k in range(KI):
            nc.gpsimd.indirect_dma_start(
                out=gth[:, k, :],
                out_offset=None,
                in_=xs_in[:, :],
                in_offset=bass.IndirectOffsetOnAxis(ap=idxIs[:, k:k + 1], axis=0),
            )
        fmS = []
        for h in range(2):
            t = big.tile([128, HWP], F32, tag=f"fmS{h}", name=f"fmS{h}")
            fmS.append(t)
        for k in range(KI):
            for h in range(2):
                ps = psT.tile([128, 128], F32, tag="psT")
                nc.tensor.transpose(ps, gth[:, k, h * 128:(h + 1) * 128], ident)
                nc.scalar.activation(
                    fmS[h][:, k * 128:(k + 1) * 128], ps,
                    mybir.ActivationFunctionType.Copy, scale=INV_EPS1,
                )
        for h in range(2):
            nc.sync.dma_start(out=fmo_out[h], in_=fmS[h][:, :HW])

        # ---- phase A: tokens_sum = OH^T @ fmT_hi + OH^T @ [fmT_lo | 1] ----
        # 3 rounds of <=3 n-tiles; each n-tile holds 2 PSUM banks (hi, lo+cnt)
        # so max concurrent banks = 6 (+2 for the transposes above).
        tokS = big.tile([128, 7, C], F32, tag="tokS")
        for (r0, r1) in [(0, 3), (3, 6), (6, 7)]:
            psums = {}
            for m in range(r0, r1):
                nm = MT[m]
                ph = psA.tile([nm, C], F32, tag=f"psH{m % 3}", name=f"psH{m}")
                pl = psA.tile([nm, C + 1], F32, tag=f"psL{m % 3}", name=f"psL{m}")
                psums[m] = (ph, pl)
            for k in range(KI):
                for m in range(r0, r1):
                    nm = MT[m]
                    ph, pl = psums[m]
                    lhs = ohA[k][:, MSTART[m]:MSTART[m] + nm]
                    nc.tensor.matmul(
                        ph, lhsT=lhs, rhs=fthi[:, k, :],
                        start=(k == 0), stop=(k == KI - 1),
                    )
                    nc.tensor.matmul(
                        pl, lhsT=lhs, rhs=ftlo[:, k, :],
                        start=(k == 0), stop=(k == KI - 1),
                    )
            # epilogue per n-tile: tokens = (hi + lo) / (cnt + 1e-6)
            for m in range(r0, r1):
                nm = MT[m]
                ph, pl = psums[m]
                tl = tokL_pool.tile([128, C + 1], F32, tag="tokL")
                nc.scalar.activation(
                    tl[:nm, :], pl, mybir.ActivationFunctionType.Copy
                )
                cse = epi_pool.tile([128, 1], F32, tag="cse")
                nc.vector.tensor_scalar(
                    cse[:nm, :], tl[:nm, C:C + 1], 1e-6, None,
                    op0=mybir.AluOpType.add,
                )
                rcp = epi_pool.tile([128, 1], F32, tag="rcp")
                nc.vector.reciprocal(rcp[:nm, :], cse[:nm, :])
                tmp = epi_pool.tile([128, C], F32, tag="tmp")
                nc.vector.tensor_tensor(
                    tmp[:nm, :], ph, tl[:nm, :C], op=mybir.AluOpType.add
                )
                nc.vector.tensor_scalar(
                    tokS[:nm, m, :], tmp[:nm, :], rcp[:nm, :], None,
                    op0=mybir.AluOpType.mult,
                )
                nc.sync.dma_start(
                    out=tok_out[MSTART[m]:MSTART[m] + nm, :], in_=tokS[:nm, m, :]
                )


def _prep_core_inputs(fm_b, x_b, idx_b):
    fmT = np.zeros((HWP, C), dtype=np.float32)
    fmT[:HW] = np.ascontiguousarray(fm_b.reshape(C, HW), dtype=np.float32).T
    fthi = fmT.astype(np.float16)
    lo = (fmT - fthi.astype(np.float32)).astype(np.float16)
    ftlo = np.ones((HWP, C + 1), dtype=np.float16)
    ftlo[:, :C] = lo
    xs = np.ascontiguousarray(x_b, dtype=np.float32)
    idx = np.asarray(idx_b)
    idxT = np.full(HWP, SENTINEL, dtype=np.float32)
    idxT[:HW] = idx.astype(np.float32)
    idxT = np.ascontiguousarray(idxT.reshape(KI, 128).T)
    idxI = np.zeros(HWP, dtype=np.int32)
    idxI[:HW] = idx.astype(np.int32)
    idxI = np.ascontiguousarray(idxI.reshape(KI, 128).T)
    return {"fthi": np.ascontiguousarray(fthi), "ftlo": ftlo,
            "xs": xs, "idxT": idxT, "idxI": idxI}


def kernel(feature_map, x, idx_token, token_num=N, map_H=56, map_W=56,
           init_H=56, init_W=56, **_ignored):
    from concourse.bass_utils import run_bass_kernel_spmd

    feature_map = np.asarray(feature_map)
    x = np.asarray(x)
    idx_token = np.asarray(idx_token)
    assert feature_map.shape == (B, C, 56, 56), feature_map.shape
    assert x.shape == (B, N, C), x.shape
    assert idx_token.shape == (B, HW), idx_token.shape

    if "nc" not in _CACHE:
        _CACHE["nc"] = build_program()
    nc = _CACHE["nc"]

    in_maps = [
        _prep_core_inputs(feature_map[b], x[b], idx_token[b]) for b in range(B)
    ]
    res = run_bass_kernel_spmd(nc, in_maps, core_ids=list(range(B)))
    tokens = np.stack([np.asarray(res.results[b]["tok"]) for b in range(B)])
    fmap = np.stack(
        [np.asarray(res.results[b]["fmo"]).reshape(C, 56, 56) for b in range(B)]
    )
    return tokens.astype(np.float32), fmap.astype(np.float32)


# revision 12
# speedup vs baseline: 71.8147x; 71.8147x over previous
"""Trainium2 Bass kernel for CTM segment_reduce (map2token + token2map).

kernel(**inputs) takes the full arrays, shards batch B=8 across the 8
NeuronCores (one batch element per core), runs a Bass/Tile kernel, and
gathers the full outputs.

Math (H=W=H_init=W_init=56 makes the grid-index map the identity), per
batch element b with idx = idx_token[b] in [0,784)^3136:
  tokens[b, n, c] = sum_{i: idx[i]=n} fm[b, c, i] / (count_n + 1e-6)
  fmap[b, c, i]   = x[b, idx[i], c] / (1 + 1e-6)

Device strategy per core:
  - tokens (scatter-add): one-hot matmul on TensorE. One-hot tiles
    OH[i,n] = (idx[i]==n) are built on-chip (iota + is_equal) in fp16;
    fm arrives pre-transposed and split hi/lo in fp16 (hi+lo sums are
    exact to ~2^-22 relative; fp32r was measured to round to 12 mantissa
    bits, too lossy). Counts ride along as a ones column in the lo pass;
    PSUM (fp32) accumulates over all 25 i-tiles.
  - fmap (gather): indirect DMA row-gather of x (exact fp32) +
    PE transpose-mode to produce the c-major output layout.
"""

import copy
import os
import sys

import numpy as np

for _p in ("/opt/trn_rl_repo", "/root/.axon_site/_ro/trn_rl_repo"):
    if os.path.isdir(_p) and _p not in sys.path:
        sys.path.append(_p)

import concourse.bass as bass
import concourse.mybir as mybir
from concourse import library_config
from concourse.masks import make_identity
from concourse.tile import TileContext

F32 = mybir.dt.float32
F16 = mybir.dt.float16
I32 = mybir.dt.int32

# Problem constants (hardcoded per contract)
B = 8
C = 256
HW = 3136           # 56*56 positions
N = 784             # tokens
HWP = 3200          # positions padded to 25*128
KI = 25             # i-tiles of 128
MT = [128, 128, 128, 128, 128, 128, 16]   # n-tile sizes (784 = 6*128+16)
MSTART = [0, 128, 256, 384, 512, 640, 768]
SENTINEL = 9999.0
INV_EPS1 = float(np.float32(1.0) / np.float32(1.0 + 1e-6))
GCH = 5             # gather i-tiles per indirect DMA

_CACHE = {}


def legalize_multiwait(nc):
    """This walrus build allows a limited number of sync waits per
    instruction; split extras onto single-wait EventSemaphore carriers
    inserted just before the offending instruction (same engine)."""
    template = None
    for f in nc.m.functions:
        for b in f.blocks:
            for ins in b.instructions:
                if type(ins).__name__ == "InstEventSemaphore":
                    template = ins
                    break
            if template:
                break
        if template:
            break
    assert template is not None, "no EventSemaphore template found"
    cnt = 0
    for f in nc.m.functions:
        for b in f.blocks:
            insts = list(b.instructions)
            out = []
            changed = False
            for ins in insts:
                si = ins.sync_info
                if si is not None and si.on_wait and len(si.on_wait) > 1:
                    waits = list(si.on_wait)
                    for w in waits[:-1]:
                        nop = copy.deepcopy(template)
                        nop.name = f"waitsplit_{cnt}"
                        cnt += 1
                        nop.engine = ins.engine
                        nop.sync_info = mybir.SyncInfo(on_wait=[w], on_update=[])
                        try:
                            nop.set_dependency_edges([])
                        except Exception:
                            pass
                        nc.register_instruction(nop, overwrite=True)
                        out.append(nop)
                    ins.sync_info = mybir.SyncInfo(
                        on_wait=[waits[-1]], on_update=list(si.on_update or [])
                    )
                    changed = True
                out.append(ins)
            if changed:
                b.instructions.clear()
                for i in out:
                    b.add_instruction(i)
    return cnt


def build_program(reps=1):
    nc = bass.Bass("TRN2", target_bir_lowering=False, debug=False, num_swdge_queues=4)

    fthi_in = nc.declare_dram_parameter("fthi", [HWP, C], F16, isOutput=False)
    ftlo_in = nc.declare_dram_parameter("ftlo", [HWP, C + 1], F16, isOutput=False)
    xs_in = nc.declare_dram_parameter("xs", [N, C], F32, isOutput=False)
    idxT_in = nc.declare_dram_parameter("idxT", [128, KI], F32, isOutput=False)
    idxI_in = nc.declare_dram_parameter("idxI", [128, KI], I32, isOutput=False)
    tok_out = nc.declare_dram_parameter("tok", [N, C], F32, isOutput=True)
    fmo_out = nc.declare_dram_parameter("fmo", [2, 128, HW], F32, isOutput=True)

    with TileContext(nc) as tc:
        for r in range(reps):
            _emit(nc, tc, fthi_in, ftlo_in, xs_in, idxT_in, idxI_in, tok_out,
                  fmo_out, sfx=f"r{r}" if reps > 1 else "")
    legalize_multiwait(nc)
    return nc


def _emit(nc, tc, fthi_in, ftlo_in, xs_in, idxT_in, idxI_in, tok_out, fmo_out, sfx=""):
    from contextlib import ExitStack

    ctx = ExitStack()
    with ctx:
        const = ctx.enter_context(tc.tile_pool(name="const" + sfx, bufs=1))
        big = ctx.enter_context(tc.tile_pool(name="big" + sfx, bufs=1))
        tokL_pool = ctx.enter_context(tc.tile_pool(name="tokL" + sfx, bufs=2))
        epi_pool = ctx.enter_context(tc.tile_pool(name="epi" + sfx, bufs=2))
        psT = ctx.enter_context(tc.tile_pool(name="psT" + sfx, bufs=2, space="PSUM"))
        psA = ctx.enter_context(tc.tile_pool(name="psA" + sfx, bufs=1, space="PSUM"))

        # ---- input DMAs (small idx tensors first; fm chunked for early PE) ----
        idxTs = const.tile([128, KI], F32, tag="idxTs")
        nc.sync.dma_start(out=idxTs, in_=idxT_in[:, :])
        idxIs = const.tile([128, KI], I32, tag="idxIs")
        nc.sync.dma_start(out=idxIs, in_=idxI_in[:, :])
        fthi_c, ftlo_c = [], []
        fthi_r = fthi_in[:, :].rearrange("(j k p) c -> j p k c", j=5, k=5, p=128)
        ftlo_r = ftlo_in[:, :].rearrange("(j k p) c -> j p k c", j=5, k=5, p=128)
        for j in range(5):
            th = big.tile([128, 5, C], F16, tag=f"fthi{j}", name=f"fthi{j}")
            nc.sync.dma_start(out=th, in_=fthi_r[j])
            fthi_c.append(th)
            tl = big.tile([128, 5, C + 1], F16, tag=f"ftlo{j}", name=f"ftlo{j}")
            nc.scalar.dma_start(out=tl, in_=ftlo_r[j])
            ftlo_c.append(tl)

        # ---- constants ----
        ident = const.tile([128, 128], F32, tag="ident")
        make_identity(nc, ident)
        iotaN_i = const.tile([128, N], I32, tag="iotaN_i")
        nc.gpsimd.iota(iotaN_i, pattern=[[1, N]], base=0, channel_multiplier=0)
        iotaN = const.tile([128, N], F32, tag="iotaN")
        nc.gpsimd.tensor_copy(iotaN, iotaN_i)

        # ---- one-hot tiles OH[i, n] (fp16; all 25 kept resident) ----
        ohA = []
        for k in range(KI):
            oh = const.tile([128, N], F16, tag=f"ohA{k}", name=f"ohA{k}")
            nc.vector.tensor_scalar(
                oh, iotaN, idxTs[:, k:k + 1], None, op0=mybir.AluOpType.is_equal
            )
            ohA.append(oh)

        # ---- phase B: row-gather of x (exact f32) + PE transpose ----
        # the walrus build here rejects the bulk dma_gather ucode path and
        # multi-offset indirect DMAs return garbage on HW, so: 25 indirect
        # DMAs, one row per partition each (128 x 1KB)
        gth = big.tile([128, KI, C], F32, tag="gth")
        for k in range(KI):
            nc.gpsimd.indirect_dma_start(
                out=gth[:, k, :],
                out_offset=None,
                in_=xs_in[:, :],
                in_offset=bass.IndirectOffsetOnAxis(ap=idxIs[:, k:k + 1], axis=0),
            )
        fmS = []
        for h in range(2):
            t = big.tile([128, HWP], F32, tag=f"fmS{h}", name=f"fmS{h}")
            fmS.append(t)
        for k in range(KI):
            for h in range(2):
                ps = psT.tile([128, 128], F32, tag="psT")
                nc.tensor.transpose(ps, gth[:, k, h * 128:(h + 1) * 128], ident)
                nc.scalar.activation(
                    fmS[h][:, k * 128:(k + 1) * 128], ps,
                    mybir.ActivationFunctionType.Copy, scale=INV_EPS1,
                )
        for h in range(2):
            nc.sync.dma_start(out=fmo_out[h], in_=fmS[h][:, :HW])

        # ---- phase A: tokens_sum = OH^T @ fmT_hi + OH^T @ [fmT_lo | 1] ----
        # 3 rounds of <=3 n-tiles; each n-tile holds 2 PSUM banks (hi, lo+cnt)
        # so max concurrent banks = 6 (+2 for the transposes above).
        tokS = big.tile([128, 7, C], F32, tag="tokS")
        for (r0, r1) in [(0, 3), (3, 6), (6, 7)]:
            psums = {}
            for m in range(r0, r1):
                nm = MT[m]
                ph = psA.tile([nm, C], F32, tag=f"psH{m % 3}", name=f"psH{m}")
                pl = psA.tile([nm, C + 1], F32, tag=f"psL{m % 3}", name=f"psL{m}")
                psums[m] = (ph, pl)
            for k in range(KI):
                for m in range(r0, r1):
                    nm = MT[m]
                    ph, pl = psums[m]
                    lhs = ohA[k][:, MSTART[m]:MSTART[m] + nm]
                    nc.tensor.matmul(
                        ph, lhsT=lhs, rhs=fthi_c[k // 5][:, k % 5, :],
                        start=(k == 0), stop=(k == KI - 1),
                    )
                    nc.tensor.matmul(
                        pl, lhsT=lhs, rhs=ftlo_c[k // 5][:, k % 5, :],
                        start=(k == 0), stop=(k == KI - 1),
                    )
            # epilogue per n-tile: tokens = (hi + lo) / (cnt + 1e-6)
            for m in range(r0, r1):
                nm = MT[m]
                ph, pl = psums[m]
                tl = tokL_pool.tile([128, C + 1], F32, tag="tokL")
                nc.scalar.activation(
                    tl[:nm, :], pl, mybir.ActivationFunctionType.Copy
                )
                cse = epi_pool.tile([128, 1], F32, tag="cse")
                nc.vector.tensor_scalar(
                    cse[:nm, :], tl[:nm, C:C + 1], 1e-6, None,
                    op0=mybir.AluOpType.add,
                )
                rcp = epi_pool.tile([128, 1], F32, tag="rcp")
                nc.vector.reciprocal(rcp[:nm, :], cse[:nm, :])
                tmp = epi_pool.tile([128, C], F32, tag="tmp")
                nc.vector.tensor_tensor(
                    tmp[:nm, :], ph, tl[:nm, :C], op=mybir.AluOpType.add
                )
                nc.vector.tensor_scalar(
                    tokS[:nm, m, :], tmp[:nm, :], rcp[:nm, :], None,
                    op0=mybir.AluOpType.mult,
                )
                nc.sync.dma_start(
                    out=tok_out[MSTART[m]:MSTART[m] + nm, :], in_=tokS[:nm, m, :]
                )


def _prep_core_inputs(fm_b, x_b, idx_b):
    fmT = np.zeros((HWP, C), dtype=np.float32)
    fmT[:HW] = np.ascontiguousarray(fm_b.reshape(C, HW), dtype=np.float32).T
    fthi = fmT.astype(np.float16)
    lo = (fmT - fthi.astype(np.float32)).astype(np.float16)
    ftlo = np.ones((HWP, C + 1), dtype=np.float16)
    ftlo[:, :C] = lo
    xs = np.ascontiguousarray(x_b, dtype=np.float32)
    idx = np.asarray(idx_b)
    idxT = np.full(HWP, SENTINEL, dtype=np.float32)
    idxT[:HW] = idx.astype(np.float32)
    idxT = np.ascontiguousarray(idxT.reshape(KI, 128).T)
    idxI = np.zeros(HWP, dtype=np.int32)
    idxI[:HW] = idx.astype(np.int32)
    idxI = np.ascontiguousarray(idxI.reshape(KI, 128).T)
    return {"fthi": np.ascontiguousarray(fthi), "ftlo": ftlo,
            "xs": xs, "idxT": idxT, "idxI": idxI}


def kernel(feature_map, x, idx_token, token_num=N, map_H=56, map_W=56,
           init_H=56, init_W=56, **_ignored):
    from concourse.bass_utils import run_bass_kernel_spmd

    feature_map = np.asarray(feature_map)
    x = np.asarray(x)
    idx_token = np.asarray(idx_token)
    assert feature_map.shape == (B, C, 56, 56), feature_map.shape
    assert x.shape == (B, N, C), x.shape
    assert idx_token.shape == (B, HW), idx_token.shape

    if "nc" not in _CACHE:
        _CACHE["nc"] = build_program()
    nc = _CACHE["nc"]

    in_maps = [
        _prep_core_inputs(feature_map[b], x[b], idx_token[b]) for b in range(B)
    ]
    res = run_bass_kernel_spmd(nc, in_maps, core_ids=list(range(B)))
    tokens = np.stack([np.asarray(res.results[b]["tok"]) for b in range(B)])
    fmap = np.stack(
        [np.asarray(res.results[b]["fmo"]).reshape(C, 56, 56) for b in range(B)]
    )
    return tokens.astype(np.float32), fmap.astype(np.float32)
